# revision 53
# baseline (speedup 1.0000x reference)
"""CoNystromAttention Trainium2 kernel, v5.

Shard: 8 cores = 4 batches x 2 head-groups (8 heads each). Per core:
one batch b, 8 heads organized as 4 "pairs" (2 heads = 128 partitions).

Math (reference, with Q=K=V=QKV):
  QKV = X[b].T @ Wq[h].T (+ bq folded downstream)       [n=4096, d=64]
  Qt  = window-mean(QKV, 64) + bq                       [m=64, d]
  S   = exp((QKV + bq) @ Qt.T / 8)                      [n, m]
  G   = exp(Qt @ Qt.T / 8); GD = G/rowsum(G)
  V6  = newton_schulz(GD, 6)  (per-head init scale)
  out = diag(1/r) S V6 diag(1/c) (S.T (QKV + bq)),  r/c = row/col sums

v5 highlights:
  - fp16 working dtype everywhere (rel err ~2e-3 vs bf16's 1.3e-2)
  - projection in fp8e4m3 hi/lo split with DoubleRow perf mode:
    QKV = Xh.Wh + (Xl.Wh + Xh.Wl)/16 -> 3 double-rate passes = 75% of
    the bf16 PE cost; X DMA halves to 8MB. hi/lo stacked in one dram
    tensor, hi half DMA'd first so matmuls start early
  - landmark pooling moved before the projection: window-sums of X
    (host-side; pooling commutes with the linear proj) are projected
    on-device with the same fp8 weights; the whole Gamma/Newton-
    Schulz init runs overlapped with the projection c-loop
  - bias folded into landmarks (blkq), the S^T exp (per-landmark
    activation bias b.k~/8), and dvp (+1 (x) b ones-outer term)
  - r sums via tiny PE matmuls into one PSUM bank (one reciprocal at
    the end); c via the activation accumulator on the S^T exp
  - engine balance: Act scales ps_lo->fp16 (hw allows one PSUM input
    per DVE op) and DVE adds; token-major QKV transposed during phase
    1 (PE idle-free), token-major S from PE transposes of st in phase
    2; batched [128,4,128] Newton-Schulz spread over DVE/Act; final
    diag(1/r) scale split 3 ways (DVE / Act+DVE-fp16 / Act+Pool);
    out DMA batched per 256 tokens
  - PSUM start_tensor_calc marks its whole 2KB bank lazily pending-
    zero on real hw: accumulation groups stay consecutive per bank,
    single start on the first write of each group
"""

import numpy as np

P = 128
N_TOK = 4096
EMBED = 1024
NPAIR = 4            # head-pairs per core (8 heads)
KO = EMBED // P      # 8 contraction 128-chunks (= 4 DoubleRow 256-chunks)
KC = 4               # DoubleRow 256-wide contraction chunks
XCH = 512            # phase-1 token chunk
NCH = N_TOK // XCH   # 8 projection chunks
CH = 8               # phase-2 512-token chunks
TPC = 4              # 128-token tiles per 512 chunk
NS_ITERS = 6

_CACHE = {}
_DEBUG = False


def _build():
    import concourse.mybir as mybir
    from concourse import bacc
    from concourse.tile import TileContext
    from concourse.masks import make_identity

    f32 = mybir.dt.float32
    f16 = mybir.dt.float16
    f8 = mybir.dt.float8e4
    ALU = mybir.AluOpType
    ACTF = mybir.ActivationFunctionType
    AX = mybir.AxisListType
    DR = mybir.MatmulPerfMode.DoubleRow

    nc = bacc.Bacc("TRN2", target_bir_lowering=False, debug=False)
    X8 = nc.dram_tensor("X8", [2 * EMBED, N_TOK], f8, kind="ExternalInput")
    XG8 = nc.dram_tensor("XG8", [2 * EMBED, 64], f8, kind="ExternalInput")
    W8 = nc.dram_tensor("W8", [2 * EMBED, 512], f8, kind="ExternalInput")
    bias = nc.dram_tensor("bias", [512], f32, kind="ExternalInput")
    out_d = nc.dram_tensor("out", [N_TOK, 512], f16, kind="ExternalOutput")
    dbg = {}
    if _DEBUG:
        dbg["qkvt"] = nc.dram_tensor("dbg_qkvt", [P, NPAIR * N_TOK], f16,
                                     kind="ExternalOutput")
        dbg["st"] = nc.dram_tensor("dbg_st", [P, NPAIR * N_TOK], f16,
                                   kind="ExternalOutput")
        dbg["blkq"] = nc.dram_tensor("dbg_blkq", [P, NPAIR * P], f16,
                                     kind="ExternalOutput")
        dbg["blm"] = nc.dram_tensor("dbg_blm", [P, NPAIR], f32,
                                    kind="ExternalOutput")
        dbg["wpad"] = nc.dram_tensor("dbg_wpad", [P, NPAIR * P], f16,
                                     kind="ExternalOutput")
        dbg["dvp"] = nc.dram_tensor("dbg_dvp", [P, NPAIR * P], f16,
                                    kind="ExternalOutput")
        dbg["rvr"] = nc.dram_tensor("dbg_rvr", [P, 32 * NPAIR * 2], f16,
                                    kind="ExternalOutput")
        dbg["gd4"] = nc.dram_tensor("dbg_gd4", [P, NPAIR * P], f16,
                                    kind="ExternalOutput")
        dbg["kt4"] = nc.dram_tensor("dbg_kt4", [P, NPAIR * P], f16,
                                    kind="ExternalOutput")
        dbg["v6"] = nc.dram_tensor("dbg_v6", [P, NPAIR * P], f16,
                                   kind="ExternalOutput")

    with TileContext(nc) as tc, (
        tc.tile_pool(name="big", bufs=1)
    ) as big, tc.tile_pool(name="persist", bufs=1) as pers, tc.tile_pool(
        name="nsv", bufs=1
    ) as nsp, tc.tile_pool(name="wk", bufs=4) as wk:
        # ---------------- persistent small tiles ----------------
        # weights hi-half first so the projection c0 matmuls start early;
        # w8 lo / bias DMAs are issued after c0's X chunk (see c-loop)
        xgt = pers.tile([P, 2, KO, 64], f8, tag="xgt")
        xg_re = XG8.rearrange("(two ko p) m -> p two ko m", p=P, two=2)
        w8t = pers.tile([P, 2, KO, 512], f8, tag="w8t")
        w_re = W8.rearrange("(two ko p) hd -> p two ko hd", p=P, two=2)
        nc.sync.dma_start(w8t[:, 0, 0:2], w_re[:, 0, 0:2])
        nc.sync.dma_start(w8t[:, 0, 2:8], w_re[:, 0, 2:8])
        bias_t = pers.tile([P, NPAIR], f32, tag="bias")
        bias_r32 = pers.tile([1, 512], f32, tag="bias_r32")
        bias16 = pers.tile([P, NPAIR], f16, tag="bias16")
        bias_r16 = pers.tile([1, 512], f16, tag="bias_r16")
        idf = pers.tile([P, P], f32, tag="idf")
        make_identity(nc, idf[:])
        id16 = pers.tile([P, P], f16, tag="id16")
        nc.vector.tensor_copy(id16[:], idf[:])
        idf4 = pers.tile([P, NPAIR, P], f16, tag="idf4")
        for p in range(NPAIR):
            nc.vector.tensor_copy(idf4[:, p, :], id16[:])
        i7_4 = pers.tile([P, NPAIR, P], f16, tag="i7_4")
        i15_4 = pers.tile([P, NPAIR, P], f16, tag="i15_4")
        i13_4 = pers.tile([P, NPAIR, P], f16, tag="i13_4")
        for t_, v_ in ((i7_4, 7.0), (i15_4, 15.0), (i13_4, 13.0)):
            nc.vector.tensor_scalar_mul(t_[:], idf4[:], v_)
        ones1p = pers.tile([1, P], f16, tag="ones1p")
        nc.vector.memset(ones1p[:], 1.0)
        ones2 = pers.tile([P, 2], f16, tag="ones2")
        nc.vector.memset(ones2[:], 0.0)
        nc.vector.memset(ones2[0:64, 0:1], 1.0)
        nc.vector.memset(ones2[64:128, 1:2], 1.0)
        ones128 = pers.tile([P, 1], f16, tag="ones128")
        nc.vector.memset(ones128[:], 1.0)

        qkvt = big.tile([P, NPAIR, N_TOK], f16, tag="qkvt")
        qnb_all = big.tile([P, 32, 512], f16, tag="qnb_all")
        st = big.tile([P, NPAIR, N_TOK], f16, tag="st")
        rvr16 = pers.tile([P, 32, NPAIR, 2], f16, tag="rvr16")
        wpad = pers.tile([P, NPAIR, P], f16, tag="wpad")
        nc.vector.memset(wpad[:], 0.0)

        with tc.tile_pool(name="nsps", bufs=2, space="PSUM") as nsps:
            # ============ phase 1: projection c-loop, with the Gamma /
            # Newton-Schulz init interleaved at chunk boundaries ==========
            def interlude_a():
                # blm[lm, p] = (blkq^T bq)/8 : per-landmark exp bias
                blmps = nsps.tile([P, NPAIR], f32, tag="nsb", name="blmps")
                for p in range(NPAIR):
                    nc.tensor.matmul(
                        blmps[:, p:p + 1], blkq4[:, p, :], bias16[:, p:p + 1],
                        start=True, stop=True,
                        skip_group_check=True,
                    )
                nc.vector.tensor_scalar_mul(blm[:], blmps[:], 0.125)
                psg4 = nsps.tile([P, NPAIR, P], f32, tag="nsb", name="psg4")
                for p in range(NPAIR):
                    nc.tensor.matmul(
                        psg4[:, p, :], blkq4[:, p, :], blkq4[:, p, :],
                        start=True, stop=True,
                        skip_group_check=True,
                    )
                gs = wk.tile([P, NPAIR], f32, tag="gs")
                for p in range(NPAIR):
                    nc.scalar.activation(
                        g4[:, p, :], psg4[:, p, :], ACTF.Exp, scale=0.125,
                        accum_out=gs[:, p:p + 1],
                    )
                # zero exp(0)=1 off-blocks; fix accumulated rowsums
                nc.vector.memset(g4[0:64, :, 64:128], 0.0)
                nc.vector.memset(g4[64:128, :, 0:64], 0.0)
                gs2 = wk.tile([P, NPAIR], f32, tag="gs2")
                nc.vector.tensor_scalar_add(gs2[:], gs[:], -64.0)
                nc.vector.reciprocal(gri[:], gs2[:])
                gri_b = gri[:].rearrange("p (f o) -> p f o", o=1).to_broadcast(
                    [P, NPAIR, P])
                nc.vector.tensor_tensor(gd4[:], g4[:], gri_b, ALU.mult)
                # K^T = GD^T = G @ diag(gri)  (G symmetric)
                nc.vector.tensor_tensor(dmat4[:], idf4[:], gri_b, ALU.mult)

            def interlude_b():
                ktps = nsps.tile([P, NPAIR, P], f32, tag="nsb", name="ktps")
                for p in range(NPAIR):
                    nc.tensor.matmul(
                        ktps[:, p, :], g4[:, p, :], dmat4[:, p, :],
                        start=True, stop=True,
                        skip_group_check=True,
                    )
                nc.scalar.copy(kt4[:], ktps[:])
                # per-head max colsum -> scale (rowsums of GD are 1).
                # head-major order so sv4 extraction is contiguous.
                csps = nsps.tile([1, NPAIR * P], f32, tag="nsb", name="csps")
                nc.tensor.matmul(csps[:], ones128[:], gd4[:],
                                 start=True, stop=True)
                cm = wk.tile([1, 8], f32, tag="cm")
                nc.vector.reduce_max(
                    cm[:].rearrange("o (h f) -> o h f", h=2),
                    csps[:].rearrange("o (f h l) -> o h f l", h=2, l=64),
                    axis=AX.X,
                )
                with nc.allow_low_precision(reason="NS init scale"):
                    nc.vector.reciprocal(cmr[:], cm[:])

            def interlude_c():
                biasps = nsps.tile([P, NPAIR * P], f32, tag="nsb",
                                   name="biasps")
                nc.tensor.matmul(biasps[:], ones1p[:], bias_r16[:],
                                 start=True, stop=True)
                nc.scalar.copy(
                    biasfree[:], biasps[:].rearrange("p (f d) -> p f d", d=P))
                bps = nsps.tile([P, 8], f32, tag="nsb", name="bps")
                nc.tensor.matmul(bps[:], ones1p[:], cmr[:],
                                 start=True, stop=True)
                sv4 = wk.tile([P, NPAIR], f32, tag="sv4")
                nc.vector.tensor_copy(sv4[0:64, :], bps[0:64, 0:NPAIR])
                nc.vector.tensor_copy(sv4[64:128, :], bps[64:128, NPAIR:8])
                sv_b = sv4[:].rearrange("p (f o) -> p f o", o=1).to_broadcast(
                    [P, NPAIR, P])
                v0 = nsp.tile([P, NPAIR, P], f16, tag="v", name="v0")
                nc.vector.tensor_tensor(v0[:], kt4[:], sv_b, ALU.mult)
                vt0 = nsp.tile([P, NPAIR, P], f16, tag="vt", name="vt0")
                nc.vector.tensor_tensor(vt0[:], gd4[:], sv_b, ALU.mult)
                vstate[0] = v0
                vstate[1] = vt0

            blm = pers.tile([P, NPAIR], f32, tag="blm")
            biasfree = pers.tile([P, NPAIR, P], f16, tag="biasfree")
            blkq4 = pers.tile([P, NPAIR, P], f16, tag="blkq4")

            def landmarks():
                # tiny fp8 DoubleRow projection of the X window-sums
                psb_h = nsps.tile([P, NPAIR, P], f32, tag="nsb", name="psbh")
                psb_l = nsps.tile([P, NPAIR, P], f32, tag="nsb", name="psbl")
                for p in range(NPAIR):
                    psl = slice(p * P, (p + 1) * P)
                    for kc in range(KC):
                        kcs = slice(2 * kc, 2 * kc + 2)
                        nc.tensor.matmul(
                            psb_h[:, p, 0:64], w8t[:, 0, kcs, psl],
                            xgt[:, 0, kcs, :],
                            start=(kc == 0), stop=(kc == KC - 1),
                            perf_mode=DR, skip_group_check=True,
                        )
                for p in range(NPAIR):
                    psl = slice(p * P, (p + 1) * P)
                    for kc in range(KC):
                        kcs = slice(2 * kc, 2 * kc + 2)
                        nc.tensor.matmul(
                            psb_l[:, p, 0:64], w8t[:, 0, kcs, psl],
                            xgt[:, 1, kcs, :],
                            start=(kc == 0), stop=False,
                            perf_mode=DR, skip_group_check=True,
                        )
                        nc.tensor.matmul(
                            psb_l[:, p, 0:64], w8t[:, 1, kcs, psl],
                            xgt[:, 0, kcs, :],
                            start=False, stop=(kc == KC - 1),
                            perf_mode=DR, skip_group_check=True,
                        )
                # qsum = psb_h + psb_l/16 = (16 psb_h + psb_l)/16
                # (hardware: only one PSUM input per DVE op)
                pl_sb = wk.tile([P, NPAIR, 64], f32, tag="pl_sb")
                nc.scalar.copy(pl_sb[:], psb_l[:, :, 0:64])
                qs16 = wk.tile([P, NPAIR, 64], f32, tag="qs16")
                nc.vector.scalar_tensor_tensor(
                    qs16[:], psb_h[:, :, 0:64], 16.0, pl_sb[:],
                    ALU.mult, ALU.add,
                )
                # blkq = block-diag(qsum/64 + bias) = qs16/1024 + bias
                nc.vector.memset(blkq4[:], 0.0)
                bb_u = bias_t[0:64, :].rearrange("p (f o) -> p f o", o=1)
                nc.vector.scalar_tensor_tensor(
                    blkq4[0:64, :, 0:64], qs16[0:64, :, :], 1.0 / 1024,
                    bb_u.to_broadcast([64, NPAIR, 64]), ALU.mult, ALU.add,
                )
                bb_l = bias_t[64:128, :].rearrange("p (f o) -> p f o", o=1)
                nc.vector.scalar_tensor_tensor(
                    blkq4[64:128, :, 64:128], qs16[64:128, :, :], 1.0 / 1024,
                    bb_l.to_broadcast([64, NPAIR, 64]), ALU.mult, ALU.add,
                )
            g4 = nsp.tile([P, NPAIR, P], f16, tag="g4")
            gd4 = nsp.tile([P, NPAIR, P], f16, tag="gd4")
            dmat4 = nsp.tile([P, NPAIR, P], f16, tag="dmat4")
            kt4 = nsp.tile([P, NPAIR, P], f16, tag="kt4")
            gri = wk.tile([P, NPAIR], f32, tag="gri")
            cmr = wk.tile([1, 8], f16, tag="cmr")
            vstate = [None, None]
            interludes = {1: interlude_a, 3: interlude_b, 5: interlude_c}

            def qt_chunk(qsT1, c):
                # token-major QKV for chunk c's four 128-token tiles
                for t in range(TPC):
                    e = c * TPC + t
                    tsl = slice(e * P, (e + 1) * P)
                    pq = qsT1.tile([P, 512], f16, tag="q1", name=f"pq{e}")
                    for p in range(NPAIR):
                        nc.tensor.matmul(
                            pq[:, p * P:(p + 1) * P], qkvt[:, p, tsl],
                            id16[:], is_transpose=True,
                            start=True, stop=(p == NPAIR - 1),
                            skip_group_check=True,
                        )
                    nc.vector.tensor_copy(qnb_all[:, e, :], pq[:])

            with (
                tc.tile_pool(name="x", bufs=3) as xpool,
                tc.tile_pool(name="pp", bufs=2, space="PSUM") as pp,
                tc.tile_pool(name="qsT1", bufs=2, space="PSUM") as qsT1,
            ):
                x_re = X8.rearrange("(two ko p) n -> p two ko n", p=P, two=2)
                # c0 hi chunk ahead of the lo weights; bias DMAs last
                xt0 = xpool.tile([P, 2, KO, XCH], f8, tag="xt", name="xt0")
                nc.sync.dma_start(xt0[:, 0, 0:2], x_re[:, 0, 0:2, 0:XCH])
                nc.sync.dma_start(xt0[:, 0, 2:8], x_re[:, 0, 2:8, 0:XCH])
                nc.sync.dma_start(w8t[:, 1], w_re[:, 1])
                nc.sync.dma_start(xt0[:, 1], x_re[:, 1, :, 0:XCH])
                nc.sync.dma_start(xgt[:], xg_re[:])
                nc.sync.dma_start(bias_t[:],
                                  bias.rearrange("(f p) -> p f", p=P))
                nc.sync.dma_start(bias_r32[:],
                                  bias.rearrange("(o b) -> o b", o=1))
                nc.vector.tensor_copy(bias16[:], bias_t[:])
                nc.vector.tensor_copy(bias_r16[:], bias_r32[:])
                for c in range(NCH):
                    csl = slice(c * XCH, (c + 1) * XCH)
                    if c == 0:
                        xt = xt0
                    else:
                        xt = xpool.tile([P, 2, KO, XCH], f8, tag="xt",
                                        name=f"xt{c}")
                        nc.sync.dma_start(xt[:, 0], x_re[:, 0, :, csl])
                        nc.sync.dma_start(xt[:, 1], x_re[:, 1, :, csl])
                    for p in range(NPAIR):
                        psl = slice(p * P, (p + 1) * P)
                        ps_hi = pp.tile([P, XCH], f32, tag="ph",
                                        name=f"ph{c}_{p}")
                        ps_lo = pp.tile([P, XCH], f32, tag="pl",
                                        name=f"pl{c}_{p}")
                        for kc in range(KC):
                            kcs = slice(2 * kc, 2 * kc + 2)
                            # shared stationary Wh[kc]: hi + Xl cross pass
                            nc.tensor.matmul(
                                ps_hi[:], w8t[:, 0, kcs, psl],
                                xt[:, 0, kcs, :],
                                start=(kc == 0), stop=(kc == KC - 1),
                                perf_mode=DR, skip_group_check=True,
                            )
                            nc.tensor.matmul(
                                ps_lo[:], w8t[:, 0, kcs, psl],
                                xt[:, 1, kcs, :],
                                start=(kc == 0), stop=False,
                                perf_mode=DR, skip_group_check=True,
                            )
                        for kc in range(KC):
                            kcs = slice(2 * kc, 2 * kc + 2)
                            nc.tensor.matmul(
                                ps_lo[:], w8t[:, 1, kcs, psl],
                                xt[:, 0, kcs, :],
                                start=False, stop=(kc == KC - 1),
                                perf_mode=DR, skip_group_check=True,
                            )
                        # qkvt = ps_hi + ps_lo/16  (no bias!). Hardware
                        # allows one PSUM input per DVE op, so Act scales
                        # ps_lo down to fp16 first.
                        plo16 = xpool.tile([P, XCH], f16, tag="plo16",
                                           name=f"plo{c}_{p}", bufs=3)
                        nc.scalar.activation(plo16[:], ps_lo[:], ACTF.Copy,
                                             scale=1.0 / 16)
                        nc.vector.tensor_tensor(
                            qkvt[:, p, csl], ps_hi[:], plo16[:], ALU.add,
                        )
                    if c == 0:
                        landmarks()
                    elif c in interludes:
                        interludes[c]()
                    if c >= 1:
                        qt_chunk(qsT1, c - 1)
                qt_chunk(qsT1, NCH - 1)

            # ============ phase 2: S^T, transposes, M, r+c, NS stream ====
            ns_live = {}

            def ns_mm4(ps, lhs, rhs):
                for p in range(NPAIR):
                    nc.tensor.matmul(
                        ps[:, p, :], lhs[:, p, :], rhs[:, p, :],
                        start=True, stop=True,
                        skip_group_check=True,
                    )

            def ns_micro(it, s):
                v, vt = vstate
                if s == 0:
                    pskv = nsps.tile([P, NPAIR, P], f32, tag="nsb",
                                     name=f"pskv{it}")
                    pskvt = nsps.tile([P, NPAIR, P], f32, tag="nsb",
                                      name=f"pskvt{it}")
                    ns_mm4(pskv, kt4, v)
                    ns_mm4(pskvt, v, kt4)
                    kvt = nsp.tile([P, NPAIR, P], f16, tag="kvt",
                                   name=f"kvt{it}")
                    if it % 2 == 0:
                        nc.vector.tensor_copy(kvt[:], pskvt[:])
                    else:
                        nc.scalar.copy(kvt[:], pskvt[:])
                    a1 = nsp.tile([P, NPAIR, P], f16, tag="a1",
                                  name=f"a1{it}")
                    nc.vector.tensor_tensor(a1[:], i7_4[:], pskv[:],
                                            ALU.subtract)
                    ns_live["t"] = (kvt, a1)
                elif s == 1:
                    kvt, a1 = ns_live["t"]
                    psa2 = nsps.tile([P, NPAIR, P], f32, tag="nsb",
                                     name=f"psa2{it}")
                    ns_mm4(psa2, kvt, a1)
                    a3 = nsp.tile([P, NPAIR, P], f16, tag="a3",
                                  name=f"a3{it}")
                    nc.vector.tensor_tensor(a3[:], i15_4[:], psa2[:],
                                            ALU.subtract)
                    ns_live["t"] = (kvt, a3)
                elif s == 2:
                    kvt, a3 = ns_live["t"]
                    psa4 = nsps.tile([P, NPAIR, P], f32, tag="nsb",
                                     name=f"psa4{it}")
                    ns_mm4(psa4, kvt, a3)
                    a5 = nsp.tile([P, NPAIR, P], f16, tag="a5",
                                  name=f"a5{it}")
                    nc.vector.tensor_tensor(a5[:], i13_4[:], psa4[:],
                                            ALU.subtract)
                    ns_live["t"] = (a5,)
                else:
                    (a5,) = ns_live["t"]
                    if it < NS_ITERS - 1:
                        psv = nsps.tile([P, NPAIR, P], f32, tag="nsb",
                                        name=f"psv{it}")
                        ns_mm4(psv, vt, a5)
                        vn = nsp.tile([P, NPAIR, P], f16, tag="v",
                                      name=f"vn{it}")
                        nc.vector.tensor_scalar_mul(vn[:], psv[:], 0.25)
                    else:
                        vn = v
                    psvt = nsps.tile([P, NPAIR, P], f32, tag="nsb",
                                     name=f"psvt{it}")
                    ns_mm4(psvt, a5, vt)
                    vtn = nsp.tile([P, NPAIR, P], f16, tag="vt",
                                   name=f"vtn{it}")
                    nc.scalar.activation(vtn[:], psvt[:], ACTF.Copy,
                                         scale=0.25)
                    vstate[0] = vn
                    vstate[1] = vtn

            ns_sched = [(it, s) for it in range(NS_ITERS) for s in range(4)]
            ns_i = [0]

            def ns_pump(k):
                for _ in range(k):
                    if ns_i[0] < len(ns_sched):
                        ns_micro(*ns_sched[ns_i[0]])
                        ns_i[0] += 1

            with (
                tc.tile_pool(name="stps", bufs=2, space="PSUM") as stps,
                tc.tile_pool(name="qsT", bufs=2, space="PSUM") as qsT,
                tc.tile_pool(name="mps", bufs=1, space="PSUM") as mps,
                tc.tile_pool(name="rps", bufs=1, space="PSUM") as rps,
                tc.tile_pool(name="mv", bufs=3) as mvp,
            ):
                mbank = mps.tile([P, NPAIR, P], f32, tag="mb", name="mbank")
                cparts = pers.tile([P, NPAIR, CH], f32, tag="cparts")
                cs = wk.tile([P, NPAIR], f32, tag="cs")
                cinv = wk.tile([P, NPAIR], f32, tag="cinv")
                mb_prev = None
                rp_all = rps.tile([P, 32, NPAIR, 2], f32, tag="rp",
                                  name="rp_all")
                for ch in range(CH):
                    sl = slice(ch * 512, (ch + 1) * 512)
                    for p in range(NPAIR):
                        psst = stps.tile([P, 512], f32, tag="stp",
                                         name=f"st{ch}_{p}")
                        nc.tensor.matmul(
                            psst[:], blkq4[:, p, :], qkvt[:, p, sl],
                            start=True, stop=True,
                        )
                        nc.scalar.activation(
                            st[:, p, sl], psst[:], ACTF.Exp,
                            scale=0.125, bias=blm[:, p:p + 1],
                            accum_out=cparts[:, p, ch:ch + 1],
                        )
                    if ch == CH - 1:
                        nc.vector.reduce_sum(cs[:], cparts[:], axis=AX.X)
                        nc.vector.reciprocal(cinv[:], cs[:])
                    for t in range(TPC):
                        e = ch * TPC + t
                        tsl = slice(e * P, (e + 1) * P)
                        pq = qsT.tile([P, 512], f16, tag="qsT",
                                      name=f"pq{e}")
                        for p in range(NPAIR):
                            nc.tensor.matmul(
                                pq[:, p * P:(p + 1) * P],
                                st[:, p, tsl], id16[:],
                                is_transpose=True,
                                start=True, stop=(p == NPAIR - 1),
                                skip_group_check=True,
                            )
                        # token-major S
                        qs_nb = mvp.tile([P, 512], f16, tag="qsnb",
                                         name=f"qsnb{e}")
                        nc.vector.tensor_copy(qs_nb[:], pq[:])
                        for p in range(NPAIR):
                            nc.tensor.matmul(
                                rp_all[:, e, p, :], st[:, p, tsl], ones2[:],
                                start=True, stop=True,
                                skip_group_check=True,
                            )
                        # mbank lags one tile so its S-copy never stalls
                        # the PE queue head
                        if mb_prev is not None:
                            qprev, eprev = mb_prev
                            for p in range(NPAIR):
                                nc.tensor.matmul(
                                    mbank[:, p, :],
                                    qprev[:, p * P:(p + 1) * P],
                                    qnb_all[:, eprev, p * P:(p + 1) * P],
                                    start=(eprev == 0 and p == 0),
                                    stop=False,
                                    skip_group_check=True,
                                )
                        mb_prev = (qs_nb, e)
                        ns_pump(1)
                qprev, eprev = mb_prev
                for p in range(NPAIR):
                    nc.tensor.matmul(
                        mbank[:, p, :],
                        qprev[:, p * P:(p + 1) * P],
                        qnb_all[:, eprev, p * P:(p + 1) * P],
                        start=False, stop=(p == NPAIR - 1),
                        skip_group_check=True,
                    )
                ns_pump(len(ns_sched))  # flush any leftovers

                # ---- 1/r (one shot), c, dvp (+bias outer), W ----
                rv = wk.tile([P, 32, NPAIR, 2], f32, tag="rv")
                nc.vector.reciprocal(rv[:], rp_all[:, 0:32, :, :])
                nc.gpsimd.tensor_copy(rvr16[:], rv[:])
                mc = nsp.tile([P, NPAIR, P], f16, tag="mc")
                for p in range(NPAIR):
                    nc.scalar.activation(mc[:, p, :], mbank[:, p, :],
                                         ACTF.Copy, scale=cinv[:, p:p + 1])
                dvp = nsp.tile([P, NPAIR, P], f16, tag="dvp")
                nc.vector.tensor_tensor(dvp[:], mc[:], biasfree[:], ALU.add)
                wps = mps.tile([P, NPAIR, P], f32, tag="mb", name="wps")
                for p in range(NPAIR):
                    nc.tensor.matmul(
                        wps[:, p, :], vstate[1][:, p, :], dvp[:, p, :],
                        start=True, stop=True,
                        skip_group_check=True,
                    )
                # wpad pre-zeroed; copy only the in-head blocks so each
                # head's output sees just its own 64 landmarks
                nc.scalar.copy(wpad[0:64, :, 0:64], wps[0:64, :, 0:64])
                nc.scalar.copy(wpad[64:128, :, 64:128],
                               wps[64:128, :, 64:128])
                if _DEBUG:
                    nc.sync.dma_start(
                        dbg["qkvt"][:], qkvt[:].rearrange("p a b -> p (a b)"))
                    nc.sync.dma_start(
                        dbg["st"][:], st[:].rearrange("p a b -> p (a b)"))
                    nc.sync.dma_start(
                        dbg["blkq"][:],
                        blkq4[:].rearrange("p a b -> p (a b)"))
                    nc.sync.dma_start(dbg["blm"][:], blm[:])
                    nc.sync.dma_start(
                        dbg["wpad"][:], wpad[:].rearrange("p a b -> p (a b)"))
                    nc.sync.dma_start(
                        dbg["dvp"][:], dvp[:].rearrange("p a b -> p (a b)"))
                    nc.sync.dma_start(
                        dbg["rvr"][:],
                        rvr16[:].rearrange("p a b c -> p (a b c)"))
                    nc.sync.dma_start(
                        dbg["gd4"][:], gd4[:].rearrange("p a b -> p (a b)"))
                    nc.sync.dma_start(
                        dbg["kt4"][:], kt4[:].rearrange("p a b -> p (a b)"))
                    nc.sync.dma_start(
                        dbg["v6"][:],
                        vstate[0][:].rearrange("p a b -> p (a b)"))

        # ---- final: out = diag(1/r) S W ----
        with (
            tc.tile_pool(name="fps", bufs=6, space="PSUM") as fps,
            tc.tile_pool(name="otp", bufs=6) as otp,
        ):
            for pair2 in range(16):
                ot2 = otp.tile([P, 2, 512], f16, tag="ot2",
                               name=f"ot2_{pair2}")
                for t in range(2):
                    e = pair2 * 2 + t
                    tsl = slice(e * P, (e + 1) * P)
                    pso = fps.tile([P, NPAIR, P], f32, tag="fin",
                                   name=f"pso{e}")
                    for p in range(NPAIR):
                        nc.tensor.matmul(
                            pso[:, p, :], st[:, p, tsl], wpad[:, p, :],
                            start=True, stop=True,
                            skip_group_check=True,
                        )
                    rvb = rvr16[:, e, :, :].rearrange(
                        "p f (h o) -> p f h o", o=1).to_broadcast(
                        [P, NPAIR, 2, 64])
                    otv = ot2[:, t, :].rearrange(
                        "p (f h d) -> p f h d", h=2, d=64)
                    mode = 0 if e >= 30 else e % 3
                    if mode == 0:
                        nc.vector.tensor_tensor(
                            otv,
                            pso[:].rearrange("p f (h d) -> p f h d", d=64),
                            rvb, ALU.mult,
                        )
                    else:
                        o16 = wk.tile([P, 512], f16, tag="o16",
                                      name=f"o16_{e}", bufs=6)
                        nc.scalar.copy(o16[:], pso[:].rearrange(
                            "p f d -> p (f d)"))
                        o16v = o16[:].rearrange("p (f h d) -> p f h d",
                                                h=2, d=64)
                        if mode == 1:
                            nc.vector.tensor_tensor(otv, o16v, rvb, ALU.mult)
                        else:
                            nc.gpsimd.tensor_tensor(otv, o16v, rvb, ALU.mult)
                osl = slice(pair2 * 256, (pair2 + 1) * 256)
                nc.sync.dma_start(
                    out_d[osl, :].rearrange("(e p) hd -> p e hd", p=P),
                    ot2[:])

    nc.compile()
    return nc


def _get_nc():
    if "nc" not in _CACHE:
        _CACHE["nc"] = _build()
    return _CACHE["nc"]


def kernel(X, Wq, bq):
    import ml_dtypes
    from concourse.bass_utils import run_bass_kernel_spmd

    nc = _get_nc()
    B, E, n = X.shape
    H = Wq.shape[0]
    f8 = ml_dtypes.float8_e4m3

    def hilo_stack(a):
        hi = a.astype(f8)
        lo = ((a - hi.astype(np.float32)) * 16.0).astype(f8)
        return np.ascontiguousarray(np.concatenate([hi, lo], axis=0))

    # per-batch fp8 hi/lo split of X and its 64-token window sums
    xs, gsl = [], []
    for b in range(B):
        xb = np.asarray(X[b], dtype=np.float32)
        xs.append(hilo_stack(xb))
        gsl.append(hilo_stack(xb.reshape(E, 64, 64).sum(axis=2)))

    in_maps = []
    for core in range(8):
        b = core // 2
        h0 = 8 * (core % 2)
        wq_c = Wq[h0:h0 + 8]                      # [8, 64, 1024]
        wqt = np.ascontiguousarray(
            wq_c.transpose(2, 0, 1).reshape(E, 512)).astype(np.float32)
        bias_c = np.ascontiguousarray(bq[h0:h0 + 8].reshape(512))
        in_maps.append({
            "X8": xs[b],
            "XG8": gsl[b],
            "W8": hilo_stack(wqt),
            "bias": bias_c,
        })
    res = run_bass_kernel_spmd(nc, in_maps, core_ids=list(range(8)))
    out = np.empty((B, H, n, 64), dtype=np.float32)
    for core in range(8):
        b = core // 2
        h0 = 8 * (core % 2)
        oc = np.asarray(res.results[core]["out"], dtype=np.float32)
        out[b, h0:h0 + 8] = oc.reshape(n, 8, 64).transpose(1, 0, 2)
    return out
